# revision 42
# baseline (speedup 1.0000x reference)
"""Trainium2 Bass kernel for nn_LDM_5927054868953 (loss_fn).

Math (see reference):
    z1 = sum_i e^{rho_i} * S1_i * S2_i
         S1_i = sum_j exp(nu_j - mat_lr[i,j]),  mat = exp(-(dist+EPS))
    z2 = sum_e w_e (rho_i + nu_j + tau_k + dist_lr[i,j] + dist_lu[i,k])
    out = z2 - z1

Identities (same as the v1 kernel):
  * sparse-edge distances are entries of the dense distance matrices, so
    the sparse term is sum(A*dist) with A = scatter(w) built on host,
    plus a host-side bias dot product.
  * exp(nu_j - m_ij) ~= e^{nu_j}(1 - m_ij) (m <= 2e-5), so
    S1_i = C_nu - corr_i with corr_i = sum_j exp(nu_j - eps - t_ij).

v3 engine mapping (per core; j on partitions, i on the free axis):
  * TensorE (the pace-setter; everything emitted so it never stalls):
      - d2 GEMM: 5x N=512 bf16 MMs per j-block
      - rank-1 a2 add: 4 K=1 MMs packed into row groups 0/32/64/96
        (concurrent) + 1 plain
      - corr reduction: M=1 MMs packed into col groups of one PSUM bank
        (partitions 0/32/64/96 of corrA + partition 0 of corrB)
      - z2 reduction: M=1 MMs of (A.t) into corrB partitions 32/64/96
      corr/z2 MMs for j-block JB are emitted after the GEMM of JB+1
      (software pipelining) so their DVE-produced inputs are ready and
      the PE stream never blocks -> HAM stays at K=8/8.
      Accumulation: ONE start=True per bank at the very first use (a
      start clears has_written for the whole bank, so a second matrix
      phase must accumulate on top; matrix-1 corr is recovered on the
      tail as total - matrix0).
  * ScalarE: only the sqrt pass (bias=b2_j), FD=1024/512 from PSUM.
  * VectorE: Schraudolph exp (tensor_scalar bf16->int16, 4x mode):
      m' = bitcast_bf16(int16(-184.66*t + s1_j)) ~= exp(nu_j - eps - t)
      (max rel err 3.3% on a term that is ~1e-7 of the output), and the
      z2 elementwise product scr = A*t (tensor_tensor, 2x mode).
  * PSUM: d2 2x[128,1024] + 2x[128,512] + corrA + corrB = 8 banks.
"""

import os
import sys

for _p in ("/opt/trn_rl_repo", "/root/.axon_site/_ro/trn_rl_repo"):
    if os.path.isdir(_p) and _p not in sys.path:
        sys.path.insert(0, _p)

import numpy as np
import ml_dtypes

from concourse import bacc, tile, mybir
from concourse.bass_utils import run_bass_kernel_spmd

BF = ml_dtypes.bfloat16
F32 = mybir.dt.float32
BF16 = mybir.dt.bfloat16
I16 = mybir.dt.int16
AF = mybir.ActivationFunctionType
ALU = mybir.AluOpType
EPS = 1e-6

SCH_MUL = -184.6645562
SCH_ADD = 16250.368
SCH_PAD = 8000.0

FULL_CFG = dict(
    N=20000, S=4000, B=4000, D=128, E=1000000,
    ncores=8, Nloc=2500, NI=2560,
    Sr=4096, Su=4096,
)


def _build_nc(cfg):
    NI = cfg["NI"]
    Sr, Su = cfg["Sr"], cfg["Su"]
    JBr, JBu = Sr // 128, Su // 128
    NC5 = NI // 512
    assert NI == 2560

    nc = bacc.Bacc("TRN2", target_bir_lowering=False, debug=False,
                   num_devices=cfg["ncores"])

    d_lpT = nc.dram_tensor("lpT", [128, NI], BF16, kind="ExternalInput")
    d_rT2 = nc.dram_tensor("rT2", [128, Sr], BF16, kind="ExternalInput")
    d_uT2 = nc.dram_tensor("uT2", [128, Su], BF16, kind="ExternalInput")
    d_a2row4 = nc.dram_tensor("a2row4", [128, NI], BF16, kind="ExternalInput")
    d_b2r = nc.dram_tensor("b2r", [128, JBr], F32, kind="ExternalInput")
    d_b2u = nc.dram_tensor("b2u", [128, JBu], F32, kind="ExternalInput")
    d_snu = nc.dram_tensor("snu", [128, JBr], F32, kind="ExternalInput")
    d_stau = nc.dram_tensor("stau", [128, JBu], F32, kind="ExternalInput")
    d_mskr = nc.dram_tensor("mskr", [128, JBr], BF16, kind="ExternalInput")
    d_msku = nc.dram_tensor("msku", [128, JBu], BF16, kind="ExternalInput")
    d_w8 = nc.dram_tensor("w8", [128, 1024], F32, kind="ExternalInput")
    d_consts = nc.dram_tensor("consts", [128, 4], F32, kind="ExternalInput")
    d_sel = nc.dram_tensor("sel", [128, 2], F32, kind="ExternalInput")
    d_Alr = nc.dram_tensor("Alr", [JBr, 128, NI], BF16, kind="ExternalInput")
    d_Alu = nc.dram_tensor("Alu", [JBu, 128, NI], BF16, kind="ExternalInput")
    d_out = nc.dram_tensor("out", [1, 8], F32, kind="ExternalOutput")

    with tile.TileContext(nc) as tc:
        with tc.tile_pool(name="const", bufs=1) as cpool, \
             tc.tile_pool(name="ap", bufs=4) as apool, \
             tc.tile_pool(name="tp", bufs=3) as tpool, \
             tc.tile_pool(name="mp", bufs=4) as mpool, \
             tc.tile_pool(name="sp", bufs=4) as spool, \
             tc.tile_pool(name="dbig", bufs=2, space="PSUM") as dbig, \
             tc.tile_pool(name="cps", bufs=1, space="PSUM") as cps:

            def load(d, shape, dt):
                t_ = cpool.tile(shape, dt, name=d.name + "_sb")
                nc.sync.dma_start(t_[:], d.ap())
                return t_

            # matrix-0 critical path first; matrix-1-only tensors last
            lpT = load(d_lpT, [128, NI], BF16)
            rT2 = load(d_rT2, [128, Sr], BF16)
            a2row4 = load(d_a2row4, [128, NI], BF16)
            b2r = load(d_b2r, [128, JBr], F32)
            snu = load(d_snu, [128, JBr], F32)
            mskr = load(d_mskr, [128, JBr], BF16)
            uT2 = load(d_uT2, [128, Su], BF16)
            b2u = load(d_b2u, [128, JBu], F32)
            stau = load(d_stau, [128, JBu], F32)
            msku = load(d_msku, [128, JBu], BF16)
            w8 = load(d_w8, [128, 1024], F32)
            consts = load(d_consts, [128, 4], F32)
            sel = load(d_sel, [128, 2], F32)

            ones128 = cpool.tile([128, 128], BF16)
            nc.vector.memset(ones128[:], 1.0)
            ones_col = cpool.tile([128, 1], BF16)
            nc.vector.memset(ones_col[:], 1.0)

            outrow = cpool.tile([1, 8], F32)
            nc.vector.memset(outrow[:], 0.0)

            # tail staging (legal partition bases are multiples of 32):
            #   corr chunk c<4 -> partition 32c; chunk 4 -> partition 0.
            #   cols 0:1024 = matrix-0 [c | c4], 1024:2048 = totals,
            #   2048:2560 = z2 rows at partitions 32/64/96.
            red = cpool.tile([128, 2560], F32)
            nc.vector.memset(red[:], 0.0)
            col8 = cpool.tile([128, 2], F32)
            nc.vector.memset(col8[:], 0.0)

            corrA = cps.tile([128, 512], F32, name="corrA")
            corrB = cps.tile([128, 512], F32, name="corrB")

            CHUNKS = ((0, 1280, dbig), (1280, 2560, dbig))
            MATS = ((JBr, rT2, b2r, snu, mskr, d_Alr),
                    (JBu, uT2, b2u, stau, msku, d_Alu))
            pairs = [(mi, jb) for mi, m in enumerate(MATS)
                     for jb in range(m[0])]
            npairs = len(pairs)

            def emit_corr_z2(k, mm_t, scr_t, part):
                """PE reduction MMs for pair k (inputs produced earlier).

                Emission order packs concurrent col-group quads:
                [corrA c0..c3 @ groups 0-3], [corr c4 @ grp0 + z2 c0..c2 @
                groups 1-3], [z2 c3, c4 @ groups 1-2].
                has_written clears are REGION-scoped (measured: an
                all-start=False accumulator drifts across runs on stale
                PSUM), so every region's chronologically-first MM carries
                start=True.
                """
                mi, jb = pairs[k]
                mskt = MATS[mi][4]
                mmb = mm_t[:].bitcast(BF16)

                def corr_mm(c):
                    bank, bp = (corrA, 32 * c) if c < 4 else (corrB, 0)
                    nc.tensor.matmul(bank[bp:bp + 1, :],
                                     mskt[:, jb:jb + 1],
                                     mmb[:, 512 * c:512 * (c + 1)],
                                     start=(k == 0),
                                     stop=(k == npairs - 1),
                                     tile_position=(0, bp),
                                     skip_group_check=True)

                def z2_mm(c):
                    bp = 32 + 32 * (c % 3)
                    nc.tensor.matmul(corrB[bp:bp + 1, :], ones_col[:],
                                     scr_t[:, 512 * c:512 * (c + 1)],
                                     start=(k == 0 and c < 3),
                                     stop=(k == npairs - 1 and c >= 2),
                                     tile_position=(0, bp),
                                     skip_group_check=True)

                if part == 0:
                    for c in range(4):
                        corr_mm(c)
                    corr_mm(4)
                else:
                    for c in range(3):
                        z2_mm(c)
                    z2_mm(3)
                    z2_mm(4)

            def evac_corr(col0):
                for c in range(NC5):
                    bank, bp = (corrA, 32 * c) if c < 4 else (corrB, 0)
                    col = col0 + (512 if c == 4 else 0)
                    nc.vector.tensor_copy(red[bp:bp + 1, col:col + 512],
                                          bank[bp:bp + 1, :])

            pending = []  # [(k, mm tile, scr tile)] emission queue, depth 2
            for k, (mi, jb) in enumerate(pairs):
                JB, lat2, b2t, s1t, mskt, d_A = MATS[mi]
                At = apool.tile([128, NI], BF16)
                nc.sync.dma_start(At[:], d_A.ap()[jb])
                tt = tpool.tile([128, NI], BF16)
                wcol = lat2[:, jb * 128:(jb + 1) * 128]
                # Per chunk: gemm subs, then its rank-1 adds (2-packed into
                # row groups), then its sqrt — chunk 0 completes early so
                # the (saturated) ACT queue never idles waiting for it.
                RGRP = ((0, 32, 64), (96, 0, 32))
                for ci, (lo, hi, pool) in enumerate(CHUNKS):
                    d2 = pool.tile([128, hi - lo], F32, name="d2")
                    for s in range(lo, hi, 512):
                        w = min(512, hi - s)
                        nc.tensor.matmul(d2[:, s - lo:s - lo + w], wcol,
                                         lpT[:, s:s + w],
                                         start=True, stop=False,
                                         skip_group_check=True)
                    for si_, bp in enumerate(RGRP[ci]):
                        s = si_ * 512
                        w = min(512, hi - lo - s)
                        nc.tensor.matmul(d2[:, s:s + w],
                                         ones128[bp:bp + 1, :],
                                         a2row4[bp:bp + 1, lo + s:lo + s + w],
                                         start=False, stop=True,
                                         tile_position=(bp, 0),
                                         skip_group_check=True)
                    nc.scalar.activation(tt[:, lo:hi], d2[:], AF.Sqrt,
                                         bias=b2t[:, jb:jb + 1], scale=1.0)
                    # pipelined corr/z2, two pairs behind so the DVE-produced
                    # inputs are a full cycle old; both blocks after chunk 0
                    # (measured best among after-c0 / after-c1 / split)
                    if ci == 0 and len(pending) == 2:
                        ek = pending[0][0]
                        emit_corr_z2(*pending[0], part=0)
                        emit_corr_z2(*pending[0], part=1)
                        pending.pop(0)
                        if pairs[ek] == (0, JBr - 1):
                            # matrix-0 corr complete; snapshot before
                            # matrix-1's first corr MM (next iteration)
                            evac_corr(0)
                # DVE: z2 product per chunk (each starts as soon as its
                # sqrt chunk lands -> scr is ready well before the PE's
                # pipelined z2 MMs), then Schraudolph exp
                scr = spool.tile([128, NI], BF16)
                for (lo, hi, _pool) in CHUNKS:
                    nc.vector.tensor_mul(scr[:, lo:hi], At[:, lo:hi],
                                         tt[:, lo:hi])
                mm = mpool.tile([128, NI], I16)
                nc.vector.tensor_scalar(mm[:], tt[:], SCH_MUL,
                                        s1t[:, jb:jb + 1],
                                        op0=ALU.mult, op1=ALU.add)
                pending.append((k, mm, scr))
            for p in pending:
                emit_corr_z2(*p, part=0)
                emit_corr_z2(*p, part=1)
            evac_corr(1024)
            for r in range(3):
                bp = 32 + 32 * r
                nc.vector.tensor_copy(red[bp:bp + 1, 2048:2560],
                                      corrB[bp:bp + 1, :])

            # ---- tail ----
            m0 = red[:, 0:1024]
            tot = red[:, 1024:2048]
            # corr1 = m0, corr2 = tot - m0; (corr-C) products == S1*S2
            nc.vector.tensor_sub(tot, tot, m0)
            nc.vector.tensor_scalar_sub(m0, m0, consts[:, 0:1])
            nc.vector.tensor_scalar_sub(tot, tot, consts[:, 1:2])
            nc.vector.tensor_mul(m0, m0, tot)
            scr8 = cpool.tile([128, 1024], F32)
            nc.vector.scalar_tensor_tensor(
                out=scr8[:], in0=m0, scalar=1.0,
                in1=w8[:], op0=ALU.bypass, op1=ALU.mult,
                accum_out=col8[:, 0:1])
            nc.vector.tensor_scalar(scr8[:, 0:512], red[:, 2048:2560],
                                    1.0, 0.0, op0=ALU.mult, op1=ALU.add,
                                    accum_out=col8[:, 1:2])
            # z1p = sel[:,0] . col8[:,0]; z2p = sel[:,1] . col8[:,1]
            nc.tensor.matmul(corrB[0:1, 0:1], sel[:, 0:1], col8[:, 0:1],
                             start=True, stop=True, skip_group_check=True)
            nc.tensor.matmul(corrB[0:1, 1:2], sel[:, 1:2], col8[:, 1:2],
                             start=True, stop=True, skip_group_check=True)
            nc.vector.tensor_copy(outrow[0:1, 0:2], corrB[0:1, 0:2])

            nc.sync.dma_start(d_out.ap(), outrow[:])

    nc.compile()
    return nc


def _pad2(a, shape, dtype, fill=0.0):
    out = np.full(shape, fill, dtype=dtype)
    out[tuple(slice(0, s) for s in a.shape)] = a
    return out


def _mk_w8(erho_loc, NI):
    e = _pad2(erho_loc[None], (1, NI), np.float32)[0]
    w8 = np.zeros((128, 1024), np.float32)
    for c in range(4):
        w8[32 * c, 0:512] = e[512 * c:512 * (c + 1)]
    w8[0, 512:1024] = e[2048:2560]
    return w8


def _host_prep(inputs, cfg):
    N, S, B = cfg["N"], cfg["S"], cfg["B"]
    ncores, Nloc, NI = cfg["ncores"], cfg["Nloc"], cfg["NI"]
    Sr, Su = cfg["Sr"], cfg["Su"]
    JBr, JBu = Sr // 128, Su // 128

    latl = np.asarray(inputs["latent_l"], np.float32)
    latr = np.asarray(inputs["latent_r"], np.float32)
    latu = np.asarray(inputs["latent_u"], np.float32)
    rho = np.asarray(inputs["rho"], np.float32)
    nu = np.asarray(inputs["nu"], np.float32)
    tau = np.asarray(inputs["tau"], np.float32)
    w = np.asarray(inputs["weights"], np.float32)
    si = np.asarray(inputs["sparse_i"]).astype(np.int64)
    sj = np.asarray(inputs["sparse_j"]).astype(np.int64)
    sk = np.asarray(inputs["sparse_k"]).astype(np.int64)

    lp = latl + np.float32(EPS)

    def cols2d(vec, padded, fill=0.0):
        v = _pad2(vec[None], (1, padded), np.float32, fill)[0]
        return np.ascontiguousarray(v.reshape(padded // 128, 128).T)

    rT2 = _pad2((np.float32(-2.0) * latr).T, (128, Sr), BF)
    uT2 = _pad2((np.float32(-2.0) * latu).T, (128, Su), BF)
    b2r = cols2d(np.sum(latr * latr, 1, dtype=np.float32), Sr)
    b2u = cols2d(np.sum(latu * latu, 1, dtype=np.float32), Su)
    snu = cols2d(np.float32(-SCH_MUL) * (nu - np.float32(EPS))
                 + np.float32(SCH_ADD), Sr, SCH_PAD)
    stau = cols2d(np.float32(-SCH_MUL) * (tau - np.float32(EPS))
                  + np.float32(SCH_ADD), Su, SCH_PAD)
    mskr = cols2d(np.ones(S, np.float32), Sr).astype(BF)
    msku = cols2d(np.ones(B, np.float32), Su).astype(BF)

    cnu = np.float32(np.sum(np.exp(nu.astype(np.float64))))
    ctau = np.float32(np.sum(np.exp(tau.astype(np.float64))))
    biasdot = float(np.sum(w.astype(np.float64)
                           * (rho[si] + nu[sj] + tau[sk]).astype(np.float64)))
    consts = np.zeros((128, 4), np.float32)
    consts[(0, 32, 64, 96), 0] = cnu
    consts[(0, 32, 64, 96), 1] = ctau
    sel = np.zeros((128, 2), np.float32)
    sel[(0, 32, 64, 96), 0] = 1.0
    sel[(32, 64, 96), 1] = 1.0
    erho_full = np.exp(rho.astype(np.float64)).astype(np.float32)

    A_lr = np.bincount(si * S + sj, w, minlength=N * S).reshape(N, S)
    A_lu = np.bincount(si * B + sk, w, minlength=N * B).reshape(N, B)

    in_maps = []
    for c in range(ncores):
        isl = slice(c * Nloc, (c + 1) * Nloc)
        lps = lp[isl]
        a2 = _pad2(np.sum(lps * lps, 1, dtype=np.float32)[None],
                   (1, NI), np.float32)
        in_maps.append(dict(
            lpT=_pad2(lps.T, (128, NI), BF),
            rT2=rT2, uT2=uT2,
            a2row4=np.broadcast_to(a2.astype(BF), (128, NI)).copy(),
            b2r=b2r, b2u=b2u, snu=snu, stau=stau, mskr=mskr, msku=msku,
            w8=_mk_w8(erho_full[isl], NI),
            consts=consts, sel=sel,
            Alr=_pad2(A_lr[isl].T, (Sr, NI), BF).reshape(JBr, 128, NI),
            Alu=_pad2(A_lu[isl].T, (Su, NI), BF).reshape(JBu, 128, NI),
        ))
    return in_maps, biasdot


def _combine(results, biasdot):
    z1 = 0.0
    z2 = float(biasdot)
    for r in results:
        o = np.asarray(r["out"], np.float64)[0]
        z1 += o[0]
        z2 += o[1]
    return np.float32(z2 - z1)


_NC_CACHE = {}


def run_cfg(inputs, cfg, trace=False, trace_kwargs=None):
    key = tuple(sorted((k, str(v)) for k, v in cfg.items()))
    if key not in _NC_CACHE:
        _NC_CACHE[key] = _build_nc(cfg)
    nc = _NC_CACHE[key]
    in_maps, biasdot = _host_prep(inputs, cfg)
    res = run_bass_kernel_spmd(nc, in_maps, list(range(cfg["ncores"])),
                               trace=trace, **(trace_kwargs or {}))
    return _combine(res.results, biasdot), res


def kernel(**inputs):
    out, _ = run_cfg(inputs, FULL_CFG)
    return out
